# revision 31
# baseline (speedup 1.0000x reference)
"""Cached grouped-query multi-head attention on 8 Trainium2 cores — v2.

Sharding: core c -> batch b = c//2, head-half = c%2 (8 of 16 heads, 2 of 4
KV groups per core). Wq/Wk column-parallel, Wo row-parallel; the two
partial Wo products per batch are summed on the host (+ bo on host).

v2 design (vs v0 baseline):
  - all matmuls bf16 (same PE rate as f32r here, halves DMA + SBUF)
  - x, past_k, rotary transposed on the HOST -> zero device transposes
  - diagonal score tiles trimmed to their active column range
  - denominator matmuls (M=1) packed 2-per-PSUM-bank at partitions {0,32};
    reciprocal via the fast-approx DVE op per head
  - output bf16; bias bo + cross-core partial sum on host
  - software-pipelined head loop: iter i emits S(i) interleaved with
    PV(i-1) and Qproj(i+2) so the PE never stalls on the scalar exp and
    stays at the high p-state.
"""

import math
import sys

import numpy as np

sys.path.insert(0, "/opt/trn_rl_repo")

B, LQ, D = 4, 1024, 2048
H, G = 16, 4
HD = 128            # head dim
GS = H // G         # heads per group (4)
PAST = 1024
LK = PAST + LQ      # 2048
NCORES = 8
NH = 8              # local heads per core
NG = 2              # local groups per core
KSUB = D // 128     # 16 contraction subtiles over D
QC = LQ // 512      # 2 query chunks of 512
QS = LQ // 128      # 8 query subtiles of 128
KC = LK // 128      # 16 key chunks of 128
NCH = D // 512      # 4 output column chunks

_PERM = np.concatenate([np.arange(0, HD, 2), np.arange(1, HD, 2)])
_PROG_CACHE = {}


def _plan_from_mask(mask):
    """Per query-chunk: tuple of (kc, q0, midx) active key chunks.

    q0 = first active column inside the [128k x 512q] tile (trim);
    midx = index into the partial-mask stack or None if full above q0.
    """
    mT = np.asarray(mask).T  # [LK, LQ]
    plan = []
    partials = []
    for qc in range(QC):
        lst = []
        for kc in range(KC):
            t = mT[kc * 128:(kc + 1) * 128, qc * 512:(qc + 1) * 512]
            if not t.any():
                continue
            colact = t.any(axis=0)
            q0 = int(np.argmax(colact))
            assert colact[q0:].all()
            midx = None
            if not t[:, q0:].all():
                midx = len(partials)
                partials.append(np.ascontiguousarray(t, dtype=np.float32))
            lst.append((kc, q0, midx))
        assert lst[0][0] == 0 and lst[0][1] == 0, "first key chunk must be full"
        plan.append(tuple(lst))
    maskp = np.stack(partials) if partials else None
    return tuple(plan), maskp


def _build_program(plan, n_part, debug=False):
    import concourse.bacc as bacc
    import concourse.mybir as mybir
    import concourse.tile as tile

    f32 = mybir.dt.float32
    bf16 = mybir.dt.bfloat16
    AF = mybir.ActivationFunctionType
    OP = mybir.AluOpType

    nc = bacc.Bacc("TRN2", target_bir_lowering=False, debug=False,
                   num_devices=NCORES)

    xt_d = nc.dram_tensor("xt", [D, LQ], bf16, kind="ExternalInput").ap()
    wq_d = nc.dram_tensor("wq", [D, NH * HD], bf16, kind="ExternalInput").ap()
    bq_d = nc.dram_tensor("bq", [NH, HD, 1], f32, kind="ExternalInput").ap()
    wk_d = nc.dram_tensor("wk", [D, NG * HD], bf16, kind="ExternalInput").ap()
    bk_d = nc.dram_tensor("bk", [NG, HD, 1], f32, kind="ExternalInput").ap()
    wv_d = nc.dram_tensor("wv", [D, NG * HD], bf16, kind="ExternalInput").ap()
    bv_d = nc.dram_tensor("bv", [1, NG * HD], f32, kind="ExternalInput").ap()
    pkt_d = nc.dram_tensor("pkt", [NG, HD, PAST], bf16,
                           kind="ExternalInput").ap()
    pv_d = nc.dram_tensor("pv", [NG, PAST, HD], bf16,
                          kind="ExternalInput").ap()
    rot_d = nc.dram_tensor("rott", [HD // 2, LQ], f32,
                           kind="ExternalInput").ap()
    wo_d = nc.dram_tensor("wo", [NH * HD, D], bf16, kind="ExternalInput").ap()
    mp_d = None
    if n_part:
        mp_d = nc.dram_tensor("maskp", [n_part, 128, 512], bf16,
                              kind="ExternalInput").ap()
    out_d = nc.dram_tensor("out", [LQ, D], bf16, kind="ExternalOutput").ap()
    if debug:
        dbg_kt = nc.dram_tensor("dbg_kt", [128, NG, LK], bf16,
                                kind="ExternalOutput").ap()
        dbg_qt = nc.dram_tensor("dbg_qt", [128, NH, LQ], bf16,
                                kind="ExternalOutput").ap()
        dbg_at = nc.dram_tensor("dbg_at", [128, NH, LQ], bf16,
                                kind="ExternalOutput").ap()
        dbg_den = nc.dram_tensor("dbg_den", [NH, 2, 512], f32,
                                 kind="ExternalOutput").ap()
        dbg_pt = nc.dram_tensor("dbg_pt", [KC, 128, 512], bf16,
                                kind="ExternalOutput").ap()

    scl = 1.0 / math.sqrt(HD)
    # per-kc view of the plan: by_kc[kc][qc] = (q0, midx, first, last)
    by_kc = [dict() for _ in range(KC)]
    for qc in range(QC):
        n = len(plan[qc])
        for pos, (kc, q0, midx) in enumerate(plan[qc]):
            by_kc[kc][qc] = (q0, midx, pos == 0, pos == n - 1)

    with tile.TileContext(nc) as tc:
        with (
            tc.tile_pool(name="const", bufs=1) as const,
            tc.tile_pool(name="persist", bufs=1) as persist,
            tc.tile_pool(name="ptp", bufs=1) as ptp,
            tc.tile_pool(name="stage", bufs=1) as stage,
            tc.tile_pool(name="ps_s", bufs=2, space="PSUM") as ps_s_pool,
            tc.tile_pool(name="ps_pv", bufs=1, space="PSUM") as ps_pv_pool,
            tc.tile_pool(name="ps_q", bufs=1, space="PSUM") as ps_q_pool,
            tc.tile_pool(name="ps_den", bufs=1, space="PSUM") as ps_den_pool,
        ):
            # ---------------- persistent SBUF state ----------------
            KT = persist.tile([128, NG, LK], bf16)         # roped K^T
            V = [persist.tile([128, KC, HD], bf16, name=f"v{g}")
                 for g in range(NG)]
            QT = persist.tile([128, NH, LQ], bf16)         # roped Q^T
            attnT = persist.tile([128, NH, LQ], bf16)      # normalized PV
            cosF = persist.tile([128, LQ], f32)
            ssgnF = persist.tile([128, LQ], f32)           # -sin top / +sin bot

            # pt tiles: one [128, 512] bf16 per active (qc, kc), reused
            # across heads (the framework serializes WAR hazards)
            pt = {}
            for qc in range(QC):
                for (kc, q0, midx) in plan[qc]:
                    pt[(qc, kc)] = ptp.tile([128, 512], bf16,
                                            tag=f"pt{qc}_{kc}",
                                            name=f"pt{qc}_{kc}")

            # ---------------- constants (bias/mask DMAs are emitted later,
            # after the critical rotT/wk/xt transfers are in flight) --------
            bias_qk = const.tile([128, NH + NG], f32)
            bv_sb = const.tile([1, NG * HD], f32)
            bv_rep = const.tile([128, NG * HD], f32)
            ones_f = const.tile([128, 1], f32)
            nc.gpsimd.memset(ones_f, 1.0)
            ones_mat = const.tile([128, 1], bf16)
            nc.vector.tensor_copy(ones_mat, ones_f)
            mp_sb = None
            if n_part:
                mp_sb = const.tile([128, n_part, 512], bf16)

            def rope(src, dst):
                # src [128, LQ] f32 (rows = even dims then odd dims);
                # dst [128, LQ] bf16: dst = src*cosF + swap_halves(src)*ssgnF
                swp = stage.tile([128, LQ], f32, tag="swp", name="swp")
                nc.scalar.dma_start(swp[0:64], src[64:128])
                nc.scalar.dma_start(swp[64:128], src[0:64])
                t = stage.tile([128, LQ], f32, tag="ropet", name="ropet")
                nc.vector.tensor_mul(t, swp, ssgnF)
                nc.vector.tensor_mul(dst, src, cosF)
                nc.vector.tensor_tensor(dst, dst, t, OP.add)

            with (
                tc.tile_pool(name="xtp", bufs=1) as xtp,
                tc.tile_pool(name="wqp", bufs=2) as wqp,
            ):
                # ---------------- big input DMAs ----------------
                # spread across SP/DVE/scalar/gpsimd queues: one queue's
                # issue cadence (~0.65us per DMA) and serialized transfers
                # would otherwise gate the whole prologue
                rotT = xtp.tile([64, LQ], f32)
                nc.scalar.dma_start(rotT, rot_d)
                wk_t = xtp.tile([128, KSUB, NG * HD], bf16)
                nc.sync.dma_start(
                    wk_t[:, :, 0:HD],
                    wk_d.rearrange("(ko ki) m -> ki ko m", ki=128)[:, :, 0:HD])
                nc.gpsimd.dma_start(
                    wk_t[:, :, HD:],
                    wk_d.rearrange("(ko ki) m -> ki ko m", ki=128)[:, :, HD:])
                # one tile per ko block so the K/Q chains start as soon as
                # each 256KB slice lands (a single tile would gate on all 16)
                xTs = [xtp.tile([128, LQ], bf16, name=f"xt{ko}")
                       for ko in range(KSUB)]
                for ko in range(KSUB):
                    eng = nc.sync if ko % 2 == 0 else nc.scalar
                    eng.dma_start(
                        xTs[ko],
                        xt_d.rearrange("(ko ki) q -> ki ko q", ki=128)
                        [:, ko, :])
                for g in range(NG):
                    nc.gpsimd.dma_start(KT[:, g, 0:PAST], pkt_d[g])
                wv_t = xtp.tile([128, KSUB, NG * HD], bf16)
                nc.gpsimd.dma_start(
                    wv_t, wv_d.rearrange("(ko ki) m -> ki ko m", ki=128))
                for g in range(NG):
                    nc.gpsimd.dma_start(
                        V[g][:, 0:PAST // 128, :],
                        pv_d[g].rearrange("(kc ki) d -> ki kc d", ki=128))
                if n_part:
                    for i in range(n_part):
                        nc.gpsimd.dma_start(mp_sb[:, i, :], mp_d[i])

                # ---------------- rotary tables ----------------
                # freq in [0, 2pi); ScalarE Sin needs [-pi, pi]:
                #   -sin(x) = sin(x - pi);  cos(x) = 1 - 2*sin^2(x/2)
                negpi = const.tile([64, 1], f32)
                nc.gpsimd.memset(negpi, -math.pi)
                nc.scalar.activation(ssgnF[0:64], rotT, AF.Sin, bias=negpi)
                s2 = stage.tile([64, LQ], f32, tag="s2", name="s2")
                nc.scalar.activation(s2, rotT, AF.Sin, scale=0.5)
                nc.vector.tensor_mul(s2, s2, s2)
                nc.vector.tensor_scalar(cosF[0:64], s2, -2.0, 1.0,
                                        OP.mult, OP.add)
                sp = stage.tile([64, LQ], f32, tag="sp", name="sp")
                nc.vector.tensor_scalar_mul(sp, ssgnF[0:64], -1.0)
                nc.scalar.dma_start(ssgnF[64:128], sp)
                nc.scalar.dma_start(cosF[64:128], cosF[0:64])

                # bias/bv DMAs (non-critical) after the big transfers
                for h in range(NH):
                    nc.scalar.dma_start(bias_qk[:, h:h + 1], bq_d[h])
                for g in range(NG):
                    nc.scalar.dma_start(bias_qk[:, NH + g:NH + g + 1],
                                        bk_d[g])
                nc.scalar.dma_start(bv_sb, bv_d)
                nc.gpsimd.partition_broadcast(bv_rep, bv_sb)

                # ---------------- K projection + rope ----------------
                def k_proj(g):
                    psk = [ps_q_pool.tile([128, 512], f32, tag=f"q{qc}",
                                          name=f"psk{g}{qc}")
                           for qc in range(QC)]
                    for ko in range(KSUB):
                        for qc in range(QC):
                            nc.tensor.matmul(
                                psk[qc], wk_t[:, ko, g * HD:(g + 1) * HD],
                                xTs[ko][:, qc * 512:(qc + 1) * 512],
                                start=(ko == 0), stop=(ko == KSUB - 1))
                    kraw = stage.tile([128, LQ], f32, tag="raw", name="kraw")
                    for qc in range(QC):
                        nc.vector.tensor_scalar_add(
                            kraw[:, qc * 512:(qc + 1) * 512], psk[qc],
                            bias_qk[:, NH + g:NH + g + 1])
                    rope(kraw, KT[:, g, PAST:])

                # ---------------- Q projection helpers ----------------
                def qproj_alloc(h):
                    wq_t = wqp.tile([128, KSUB, HD], bf16, tag="wq",
                                    name=f"wq{h}")
                    nc.sync.dma_start(
                        wq_t, wq_d.rearrange("(ko ki) m -> ki ko m", ki=128)
                        [:, :, h * HD:(h + 1) * HD])
                    psq = [ps_q_pool.tile([128, 512], f32, tag=f"q{qc}",
                                          name=f"psq{h}{qc}")
                           for qc in range(QC)]
                    return wq_t, psq

                def qproj_groups(h, wq_t, psq):
                    gs = []
                    for ko in range(KSUB):
                        def go(ko=ko):
                            for qc in range(QC):
                                nc.tensor.matmul(
                                    psq[qc], wq_t[:, ko, :],
                                    xTs[ko][:, qc * 512:(qc + 1) * 512],
                                    start=(ko == 0), stop=(ko == KSUB - 1))
                        gs.append(go)
                    return gs

                def qproj_finish(h, psq):
                    qraw = stage.tile([128, LQ], f32, tag="raw", name="qraw")
                    for qc in range(QC):
                        nc.vector.tensor_scalar_add(
                            qraw[:, qc * 512:(qc + 1) * 512], psq[qc],
                            bias_qk[:, h:h + 1])
                    rope(qraw, QT[:, h, :])

                # prologue: K(g0), Q(0), K(g1), Q(1) — rope(g0)/rope(Q0)
                # hide under the following projection's matmuls
                k_proj(0)
                for h in range(2):
                    wq_t, psq = qproj_alloc(h)
                    for go in qproj_groups(h, wq_t, psq):
                        go()
                    qproj_finish(h, psq)
                    if h == 0:
                        k_proj(1)

                # ---------------- attention head loop ----------------
                def s_group(h, kc):
                    g = h // GS
                    for qc, (q0, midx, first, last) in sorted(
                            by_kc[kc].items()):
                        ps = ps_s_pool.tile([128, 512], f32, tag="s",
                                            name="ps_s")
                        nc.tensor.matmul(
                            ps[:, q0:], KT[:, g, kc * 128:(kc + 1) * 128],
                            QT[:, h, qc * 512 + q0:(qc + 1) * 512],
                            start=True, stop=True)
                        ptt = pt[(qc, kc)]
                        nc.scalar.activation(ptt[:, q0:], ps[:, q0:], AF.Exp,
                                             scale=scl)
                        if midx is not None:
                            nc.vector.tensor_mul(ptt[:, q0:], ptt[:, q0:],
                                                 mp_sb[:, midx, q0:])

                def pv_alloc(h):
                    return [ps_pv_pool.tile([128, 512], f32, tag=f"pv{qc}",
                                            name=f"pspv{h}{qc}")
                            for qc in range(QC)]

                def pv_group(h, kc, pspv):
                    g = h // GS
                    for qc, (q0, midx, first, last) in sorted(
                            by_kc[kc].items()):
                        nc.tensor.matmul(
                            pspv[qc][:, q0:], V[g][:, kc, :],
                            pt[(qc, kc)][:, q0:],
                            start=first, stop=last, skip_group_check=True)

                def den_alloc(h):
                    return [ps_den_pool.tile([1, 512], f32, tag=f"den{qc}",
                                             name=f"den{h}{qc}")
                            for qc in range(QC)]

                def den_group(kc, dps):
                    for qc, (q0, midx, first, last) in sorted(
                            by_kc[kc].items()):
                        nc.tensor.matmul(
                            dps[qc][0:1, q0:], ones_mat,
                            pt[(qc, kc)][:, q0:],
                            start=first, stop=last, skip_group_check=True)

                def recip_bcast(dps):
                    rbs = []
                    for qc in range(QC):
                        rec = stage.tile([1, 512], f32, tag=f"rec{qc}",
                                         name="rec", bufs=2)
                        nc.vector.reciprocal_approx_fast(rec, dps[qc][0:1, :])
                        rb = stage.tile([128, 512], f32, tag=f"rb{qc}",
                                        name="rb", bufs=2)
                        nc.gpsimd.partition_broadcast(rb, rec)
                        rbs.append(rb)
                    return rbs

                def scale_out(h, rbs, pspv):
                    for qc in range(QC):
                        nc.vector.tensor_mul(
                            attnT[:, h, qc * 512:(qc + 1) * 512],
                            pspv[qc], rbs[qc])

                def v_proj_groups():
                    gs = []
                    for qs in range(QS):
                        def go(qs=qs):
                            psv = ps_pv_pool.tile([128, 512], f32,
                                                  tag=f"pv{qs % 2}",
                                                  name=f"psv{qs}")
                            for ko in range(KSUB):
                                nc.tensor.matmul(
                                    psv[:, :NG * HD],
                                    xTs[ko][:, qs * 128:(qs + 1) * 128],
                                    wv_t[:, ko, :],
                                    start=(ko == 0), stop=(ko == KSUB - 1))
                            for g in range(NG):
                                nc.vector.tensor_tensor(
                                    V[g][:, PAST // 128 + qs, :],
                                    psv[:, g * HD:(g + 1) * HD],
                                    bv_rep[:, g * HD:(g + 1) * HD], OP.add)
                        gs.append(go)
                    return gs

                prev_dps = None     # den psum of head i-1
                for i in range(NH):
                    # reciprocal of head i-1's denominators first (frees the
                    # den bank early; inputs ready since end of iter i-1)
                    rbs = recip_bcast(prev_dps) if prev_dps is not None \
                        else None
                    if i == 0:
                        pv_state = None
                        pv_gs = v_proj_groups()
                    else:
                        pv_state = pv_alloc(i - 1)
                        pv_gs = [
                            (lambda kc=kc, hh=i - 1, st=pv_state:
                             pv_group(hh, kc, st))
                            for kc in range(KC)]
                    q_gs = []
                    q_fin = None
                    if i + 2 < NH:
                        wq_t, psq = qproj_alloc(i + 2)
                        q_gs = qproj_groups(i + 2, wq_t, psq)
                        q_fin = (i + 2, psq)
                    # weave; den(i) (and, in the last iter, PV(i) into the
                    # idle ps_q banks) trail the S stream by DLAG slots
                    DLAG = 5
                    dps = den_alloc(i)
                    if i == NH - 1:
                        pspv_last = [
                            ps_q_pool.tile([128, 512], f32, tag=f"q{qc}",
                                           name=f"pspvL{qc}")
                            for qc in range(QC)]
                    npv = len(pv_gs)
                    for j in range(KC + DLAG):
                        if j < KC:
                            # PV(i-1) reads pt[kc] BEFORE exp(i) overwrites
                            # it — program order defines the dependency
                            if i == 0:
                                if j % 2 == 0 and j // 2 < npv:
                                    pv_gs[j // 2]()
                            elif j < npv:
                                pv_gs[j]()
                            s_group(i, j)
                            if j < len(q_gs):
                                q_gs[j]()
                        if j >= DLAG:
                            den_group(j - DLAG, dps)
                            if i == NH - 1:
                                pv_group(i, j - DLAG, pspv_last)
                    if rbs is not None:
                        scale_out(i - 1, rbs, pv_state)
                    if q_fin is not None:
                        qproj_finish(*q_fin)
                    prev_dps = dps
                    if debug and i == 0:
                        dsb = stage.tile([1, 2, 512], f32, tag="dbgden",
                                         name="dsb")
                        for qc in range(QC):
                            nc.vector.tensor_copy(dsb[:, qc, :],
                                                  prev_dps[qc][0:1, :])
                        nc.sync.dma_start(dbg_den[0], dsb[0])
                        for (kc, q0, midx) in plan[1]:
                            nc.sync.dma_start(dbg_pt[kc][:, q0:],
                                              pt[(1, kc)][:, q0:])

                # epilogue: normalize(7) (PV(7) was woven into iter 7)
                rbs = recip_bcast(prev_dps)
                scale_out(NH - 1, rbs, pspv_last)
                if debug:
                    nc.sync.dma_start(dbg_kt, KT)
                    nc.sync.dma_start(dbg_qt, QT)
                    nc.sync.dma_start(dbg_at, attnT)

            # ---------------- output projection ----------------
            with (
                tc.tile_pool(name="wop", bufs=1) as wop,
                tc.tile_pool(name="obp", bufs=3) as obp,
            ):
                wots = []
                for ncH in range(NCH):
                    wot = wop.tile([128, NH, 512], bf16, tag=f"wo{ncH}",
                                   name=f"wo{ncH}")
                    nc.sync.dma_start(
                        wot, wo_d.rearrange("(ho hi) n -> hi ho n", hi=128)
                        [:, :, ncH * 512:(ncH + 1) * 512])
                    wots.append(wot)
                # qs outer / h mid / ncH inner: one stationary load of
                # attnT(h, qs) serves all 4 output column chunks
                for qs in range(QS):
                    pso = [ps_pv_pool.tile([128, 512], f32, tag="pv0",
                                           name="ps_o0"),
                           ps_pv_pool.tile([128, 512], f32, tag="pv1",
                                           name="ps_o1"),
                           ps_q_pool.tile([128, 512], f32, tag="q0",
                                          name="ps_o2"),
                           ps_q_pool.tile([128, 512], f32, tag="q1",
                                          name="ps_o3")]
                    for h in range(NH):
                        for ncH in range(NCH):
                            nc.tensor.matmul(
                                pso[ncH],
                                attnT[:, h, qs * 128:(qs + 1) * 128],
                                wots[ncH][:, h, :],
                                start=(h == 0), stop=(h == NH - 1))
                    ob = obp.tile([128, NCH, 512], bf16, tag="ob", name="ob")
                    for ncH in range(NCH):
                        if ncH % 2 == 0:
                            nc.scalar.copy(ob[:, ncH, :], pso[ncH])
                        else:
                            nc.vector.tensor_copy(ob[:, ncH, :], pso[ncH])
                    nc.sync.dma_start(
                        out_d[qs * 128:(qs + 1) * 128, :],
                        ob.rearrange("p a b -> p (a b)"))

    nc.compile()
    return nc


def _prep_in_maps(inputs, plan, maskp, n_part):
    import ml_dtypes
    c32 = lambda a: np.ascontiguousarray(a, dtype=np.float32)
    cb = lambda a: np.ascontiguousarray(a, dtype=ml_dtypes.bfloat16)
    x = np.asarray(inputs["x"])
    rot = np.asarray(inputs["rotary_freqs"])
    pk = np.asarray(inputs["past_k"])
    pv = np.asarray(inputs["past_v"])
    Wq = np.asarray(inputs["Wq"]); bq = np.asarray(inputs["bq"])
    Wk = np.asarray(inputs["Wk"]); bk = np.asarray(inputs["bk"])
    Wv = np.asarray(inputs["Wv"]); bv = np.asarray(inputs["bv"])
    Wo = np.asarray(inputs["Wo"])
    in_maps = []
    for c in range(NCORES):
        b, half = c // 2, c % 2
        h0 = half * NH
        g0 = half * NG
        wq_c = np.concatenate(
            [Wq[:, (h0 + h) * HD + _PERM] for h in range(NH)], axis=1)
        bq_c = np.stack([bq[(h0 + h) * HD + _PERM] for h in range(NH)])
        wk_c = np.concatenate(
            [Wk[:, (g0 + g) * HD + _PERM] for g in range(NG)], axis=1)
        bk_c = np.stack([bk[(g0 + g) * HD + _PERM] for g in range(NG)])
        pkt = pk[b, g0:g0 + NG][..., _PERM].transpose(0, 2, 1)  # [NG,HD,PAST]
        m = {
            "xt": cb(x[b].T),
            "wq": cb(wq_c),
            "bq": c32(bq_c[..., None]),
            "wk": cb(wk_c),
            "bk": c32(bk_c[..., None]),
            "wv": cb(Wv[:, g0 * HD:(g0 + NG) * HD]),
            "bv": c32(bv[g0 * HD:(g0 + NG) * HD][None, :]),
            "pkt": cb(pkt),
            "pv": cb(pv[b, g0:g0 + NG]),
            "rott": c32(rot.T),
            "wo": cb(Wo[h0 * HD:(h0 + NH) * HD, :]),
        }
        if n_part:
            m["maskp"] = cb(maskp)
        in_maps.append(m)
    return in_maps


def _get_program(inputs):
    plan, maskp = _plan_from_mask(np.asarray(inputs["mask"]))
    n_part = 0 if maskp is None else maskp.shape[0]
    key = plan
    if key not in _PROG_CACHE:
        _PROG_CACHE[key] = _build_program(plan, n_part)
    return _PROG_CACHE[key], plan, maskp, n_part


def _run(inputs, trace=False):
    from concourse import bass_utils

    nc, plan, maskp, n_part = _get_program(inputs)
    in_maps = _prep_in_maps(inputs, plan, maskp, n_part)

    res = bass_utils.run_bass_kernel_spmd(
        nc, in_maps, list(range(NCORES)), trace=trace,
        trace_cores=list(range(NCORES)) if trace else None)

    bo = np.asarray(inputs["bo"], dtype=np.float32)
    out = np.empty((B, LQ, D), np.float32)
    for b in range(B):
        out[b] = (res.results[2 * b]["out"].astype(np.float32)
                  + res.results[2 * b + 1]["out"].astype(np.float32)) + bo
    return out, res


def kernel(**inputs) -> np.ndarray:
    out, _ = _run(inputs, trace=False)
    return out
